# revision 17
# baseline (speedup 1.0000x reference)
"""Trainium2 Bass kernel for nn_ODEFunc_90159953478502 (MoE routing, inference path).

Math (see reference):
    logits  = x @ Wg[:256] + (t*Wg[512] + bg)      # zeros kill Wg[256:512]
    w       = softmax(logits, axis=-1)             # [B, E]
    eo_e    = tanh(x @ W1[e] + b1[e]) @ W2[e] + b2[e]
    active_e = any_b(w[b,e] > 0.01)
    out     = sum_e active_e * w[:,e,None] * eo_e  # softmax max >= 1/8 > 0.01,
                                                   # so >=1 expert always active

Sharding: expert-parallel. Core e holds the full batch plus only W1[e]/W2[e]
and computes the UNNORMALIZED partial E_e[:,None] * (tanh(x@W1[e]+b1[e]) @
W2[e]) in transposed layout ([D, B]), where E_e = exp(logit_e). Because
out = (sum_e m_e * E_e . eo_e) / S shares one softmax denominator S across
experts, normalization and the 0/1 active mask move to the host-side
unshard: each core exports its exp row (EROW), the host reconstructs
S = sum_e E_e, w = E/S, the mask, and divides once. The b2 rank-1 term
(zero here) is added host-side from a numpy gating pass when nonzero.

Device structure per core (all matmuls bf16; warm PE streams one
[128,512] matmul per ~216ns, so the PE floor is ~34 MMs/chunk * 8 chunks
~= 59us; everything else is arranged to keep PE saturated from ~3us on):
  - x arrives pre-transposed (xT [D, B] bf16) so W1/W2/Wg act as matmul lhsT
    in natural layout (out = lhsT.T @ rhs, contraction on partitions).
  - W2 and Wg are prepacked on the HOST into their exact SBUF layouts
    ([128, HT*D] / [128, DT*E]) so their DMAs are few large contiguous
    row-descriptors instead of thousands of 16-512B ones.
  - DMA issue plan (per-engine FIFO; SP + Act HWDGE rings + gpsimd SWDGE):
      sync  : x_d0_c0, x_d1_c0, w1_d0, x_d0_c1, x_d0_c23, x_d0_c47,
              batched mm2 stores, final d2=0 store
      scalar: wgx, gb, b1, w1_d1, w2, EROW, final d2=1 store
      gpsimd: x_d1_c1, x_d1_c23, x_d1_c47 (off the critical path)
    so gating(0) deps (x c0 + gating consts) land ~2.5us in, mm1(0) deps
    right behind, and no critical load queues behind a megabyte transfer.
  - a short junk-matmul warmup stream (DVE-memset source, PSUM sink shared
    with the mm2 pool) trips the PE HAM clock gate during the DMA wait so
    real matmuls run at 2.4GHz nearly from the start.
  - gating weights are column-permuted per core so the core's own expert is
    partition row 0 of the [E, B] logits: no selector matmuls needed.
  - per 512-chunk: logits^T [8,512] via 2 bf16 matmuls, ACT Exp with fused
    +gbias (no max-subtract: |logits| <= ~4) emitting bf16 into a slice of
    e_all [8, B]; chunk-pair rows are gpsimd partition-broadcast to wb
    [128,1024] bf16 (no DRAM bounce, no S-matmul, no reciprocal).
  - mm2 drains: out^T tile = psum * wb, one DVE multiply per [128,512] tile;
    stores batch 2 chunks per descriptor except the final pair (small
    descriptors keep the post-compute DMA tail short).
"""

import sys

if "/opt/trn_rl_repo" not in sys.path:
    sys.path.insert(0, "/opt/trn_rl_repo")

import numpy as np

_B, _D, _H, _E = 4096, 256, 1024, 8
_NCORES = 8
_CHUNK = 512
_NCH = _B // _CHUNK
_DT = _D // 128   # 2 d-tiles
_HT = _H // 128   # 8 h-tiles
_THRESH = 0.01
_NWARM = 12       # junk warmup matmuls

_CACHE = {}


def _build():
    import concourse.bass as bass
    import concourse.tile as tile
    import concourse.mybir as mybir
    from concourse import bacc
    from contextlib import ExitStack

    F32 = mybir.dt.float32
    BF16 = mybir.dt.bfloat16
    AF = mybir.ActivationFunctionType
    ALU = mybir.AluOpType

    nc = bacc.Bacc("TRN2", target_bir_lowering=False, debug=False)

    XT = nc.declare_dram_parameter("XT", [_D, _B], BF16, isOutput=False)
    W1E = nc.declare_dram_parameter("W1E", [_D, _H], BF16, isOutput=False)
    # host-prepacked to SBUF layout: W2P[p, hh*D+d] = W2[hh*128+p, d]
    W2P = nc.declare_dram_parameter("W2P", [128, _HT * _D], BF16, isOutput=False)
    # b1 in SBUF layout [128, HT] with gb (t*Wg[2D]+bg, permuted) as col HT
    B1GB = nc.declare_dram_parameter("B1GB", [128, _HT + 1], F32, isOutput=False)
    # host-prepacked: WGP[p, d*E+e] = Wg[d*128+p, perm[e]]
    WGP = nc.declare_dram_parameter("WGP", [128, _DT * _E], BF16, isOutput=False)
    OUTT = nc.declare_dram_parameter("OUTT", [_D, _B], BF16, isOutput=True)
    EROW = nc.declare_dram_parameter("EROW", [1, _B], BF16, isOutput=True)

    with tile.TileContext(nc) as tc, ExitStack() as ctx:
        const = ctx.enter_context(tc.tile_pool(name="const", bufs=1))
        wbp = ctx.enter_context(tc.tile_pool(name="wbp", bufs=4))
        htp = ctx.enter_context(tc.tile_pool(name="htp", bufs=24))
        op = ctx.enter_context(tc.tile_pool(name="op", bufs=4))
        pg = ctx.enter_context(tc.tile_pool(name="pg", bufs=2, space="PSUM"))
        ph = ctx.enter_context(tc.tile_pool(name="ph", bufs=4, space="PSUM"))
        po = ctx.enter_context(tc.tile_pool(name="po", bufs=2, space="PSUM"))

        # ---- PE warmup: a short junk-matmul stream with no DMA deps trips
        # the HAM clock gate (4096-cycle busy window) while inputs load, so
        # the real matmul stream starts at 2.4GHz. Source tile is memset on
        # the otherwise-idle gpsimd (no uninitialized reads); sink shares
        # the mm2 PSUM pool (its slot is recycled long after the junk
        # stream retires).
        junk = const.tile([128, 128 + _CHUNK], BF16)
        nc.gpsimd.memset(junk[:], 0.0)
        pjunk = po.tile([128, _CHUNK], F32, tag="pso")
        for _ in range(_NWARM):
            nc.tensor.matmul(
                pjunk[:], junk[:, 0:128], junk[:, 128 : 128 + _CHUNK],
                start=True, stop=True,
            )

        # ---- inputs, two phases. The 16 SDMA engines round-robin fairly
        # over every QUEUED transfer, so a critical 128KB load queued next
        # to megabytes of bulk only gets its fair share. Phase 1 queues
        # just the chunk-0/1 working set (x c0/c1, gating consts, w1) over
        # three DGE paths. Phase 2 (w2, x c2..c7) is held back by a
        # tile-visible WAW: a 1-column DVE copy from the last phase-1 tile
        # into each phase-2 destination, so its descriptors only enter the
        # rings once phase 1 has landed. -----------------------------------
        xd = [
            const.tile([128, _B], BF16, tag=f"xd_{d}", name=f"xd_{d}")
            for d in range(_DT)
        ]
        xm = {(d, c): xd[d][:, c * _CHUNK : (c + 1) * _CHUNK]
              for d in range(_DT) for c in range(_NCH)}
        w1 = [
            const.tile([128, _H], BF16, tag=f"w1_{d}", name=f"w1_{d}")
            for d in range(_DT)
        ]

        # descriptor size = per-partition contiguous bytes, and it sets the
        # effective HBM rate (~190GB/s at 1KB rows vs ~300GB/s at 2-8KB), so
        # x loads are 1024-column (2KB) pieces and w1 whole tiles (2KB).
        # phase 1 — SP ring
        nc.sync.dma_start(xd[0][:, 0:_CHUNK], XT.ap()[0:128, 0:_CHUNK])
        nc.sync.dma_start(w1[0][:], W1E.ap()[0:128, :])
        # phase 1 — Act ring (xd1 c01 leads so gating(0)'s d1 matmul and
        # Exp unblock ASAP; nothing else EVER queues DMA here before the
        # Exp/Tanh stream — a DIRECT2D is ~650ns of sequencer time and
        # stalls ACT issue)
        nc.scalar.dma_start(xd[1][:, 0:_CHUNK], XT.ap()[128:256, 0:_CHUNK])
        wgx_sb = const.tile([128, _DT * _E], BF16)
        nc.scalar.dma_start(wgx_sb[:], WGP.ap())
        b1gb = const.tile([128, _HT + 1], F32)
        nc.scalar.dma_start(b1gb[:], B1GB.ap())
        b1_sb = b1gb[:, 0:_HT]
        gb_sb = b1gb[0:_E, _HT : _HT + 1]
        nc.scalar.dma_start(w1[1][:], W1E.ap()[128:256, :])

        # phase 2 on the sync + gpsimd queues only (no early engine work
        # there), gated on the last phase-1 transfer via WAW slivers
        w2_all = const.tile([128, _HT * _D], BF16)
        gate_src = w1[1][:, _H - 1 : _H]
        for dst in (
            w2_all[:, 0:1],
            xd[0][:, _CHUNK : _CHUNK + 1],
            xd[1][:, _CHUNK : _CHUNK + 1],
            xd[0][:, 2 * _CHUNK : 2 * _CHUNK + 1],
            xd[1][:, 2 * _CHUNK : 2 * _CHUNK + 1],
            xd[0][:, _B // 2 : _B // 2 + 1],
            xd[1][:, _B // 2 : _B // 2 + 1],
        ):
            nc.vector.tensor_copy(dst, gate_src)
        nc.sync.dma_start(
            xd[0][:, _CHUNK : 2 * _CHUNK], XT.ap()[0:128, _CHUNK : 2 * _CHUNK]
        )
        nc.sync.dma_start(
            xd[0][:, 2 * _CHUNK : _B // 2],
            XT.ap()[0:128, 2 * _CHUNK : _B // 2],
        )
        nc.sync.dma_start(w2_all[:], W2P.ap())
        nc.sync.dma_start(xd[0][:, _B // 2 : _B], XT.ap()[0:128, _B // 2 : _B])
        nc.gpsimd.dma_start(
            xd[1][:, _CHUNK : 2 * _CHUNK], XT.ap()[128:256, _CHUNK : 2 * _CHUNK]
        )
        nc.gpsimd.dma_start(
            xd[1][:, 2 * _CHUNK : _B // 2],
            XT.ap()[128:256, 2 * _CHUNK : _B // 2],
        )
        nc.gpsimd.dma_start(xd[1][:, _B // 2 : _B], XT.ap()[128:256, _B // 2 : _B])

        # ---- gating chunk: unnormalized own-expert exp row -> wb broadcast.
        # Softmax normalization and the active mask move to the host-side
        # unshard: out = (sum_e m_e * (E_e . eo_e)) / S shares one
        # denominator S across experts, and each core exports its exp row
        # (EROW), from which the host reconstructs S, w, and the mask.
        e_all = const.tile([_E, _B], BF16)
        wb_tiles = {}

        def gating(c):
            psg = pg.tile([_E, _CHUNK], F32, tag="pg")
            for d in range(_DT):
                nc.tensor.matmul(
                    psg[:], wgx_sb[:, d * _E : (d + 1) * _E], xm[(d, c)],
                    start=(d == 0), stop=(d == _DT - 1),
                )
            cs = slice(c * _CHUNK, (c + 1) * _CHUNK)
            nc.scalar.activation(e_all[:, cs], psg[:], AF.Exp, bias=gb_sb[:])
            if c % 2 == 1:
                ps2 = slice((c - 1) * _CHUNK, (c + 1) * _CHUNK)
                wb = wbp.tile([128, 2 * _CHUNK], BF16, tag="wb")
                nc.gpsimd.partition_broadcast(wb[:], e_all[0:1, ps2], 128)
                wb_tiles[c - 1] = wb[:, 0:_CHUNK]
                wb_tiles[c] = wb[:, _CHUNK : 2 * _CHUNK]

        # ---- main, software-pipelined on PE: mm1(c+1) precedes mm2(c) ------
        ht_by_chunk = {}

        def mm1(c):
            ht_tiles = []
            for hh in range(_HT):
                psh = ph.tile([128, _CHUNK], F32, tag="psh")
                for d in range(_DT):
                    nc.tensor.matmul(
                        psh[:],
                        w1[d][:, hh * 128 : (hh + 1) * 128],
                        xm[(d, c)],
                        start=(d == 0), stop=(d == _DT - 1),
                    )
                ht = htp.tile([128, _CHUNK], BF16, tag="ht")
                nc.scalar.activation(
                    ht[:], psh[:], AF.Tanh, bias=b1_sb[:, hh : hh + 1]
                )
                ht_tiles.append(ht)
            ht_by_chunk[c] = ht_tiles

        obuf = {}

        def mm2(c):
            # outputs batch 2 chunks per descriptor except the final pair:
            # small per-chunk descriptors there keep the post-compute DMA
            # tail short (a trailing 1MB transfer costs ~2.6us before the
            # teardown barrier can pass).
            batch = c < _NCH - 2
            half = c % 2
            ht_tiles = ht_by_chunk.pop(c)
            for d2 in range(_DT):
                pso = po.tile([128, _CHUNK], F32, tag="pso")
                for hh in range(_HT):
                    nc.tensor.matmul(
                        pso[:],
                        w2_all[:, hh * _D + d2 * 128 : hh * _D + (d2 + 1) * 128],
                        ht_tiles[hh][:],
                        start=(hh == 0), stop=(hh == _HT - 1),
                    )
                if batch:
                    if half == 0:
                        osb_t = op.tile([128, 2 * _CHUNK], BF16, tag="osb")
                        obuf[d2] = osb_t
                    osb = obuf[d2]
                    nc.vector.tensor_tensor(
                        osb[:, half * _CHUNK : (half + 1) * _CHUNK],
                        pso[:], wb_tiles[c], ALU.mult,
                    )
                    if half == 1:
                        nc.sync.dma_start(
                            OUTT.ap()[
                                d2 * 128 : (d2 + 1) * 128,
                                (c - 1) * _CHUNK : (c + 1) * _CHUNK,
                            ],
                            osb[:],
                        )
                else:
                    osb_t = op.tile([128, _CHUNK], BF16, tag="osb1")
                    nc.vector.tensor_tensor(
                        osb_t[:], pso[:], wb_tiles[c], ALU.mult
                    )
                    eng = nc.sync if d2 == 0 else nc.scalar
                    eng.dma_start(
                        OUTT.ap()[
                            d2 * 128 : (d2 + 1) * 128,
                            c * _CHUNK : (c + 1) * _CHUNK,
                        ],
                        osb_t[:],
                    )

        gating(0)
        mm1(0)
        gating(1)
        mm1(1)
        for c in range(_NCH):
            if c + 2 < _NCH:
                gating(c + 2)
                if c + 2 == _NCH - 1:
                    # EROW only needs the last Exp; issuing it here keeps it
                    # off the ring tails behind the final stores
                    nc.scalar.dma_start(EROW.ap()[0:1, :], e_all[0:1, :])
            if c + 2 < _NCH:
                mm1(c + 2)
            mm2(c)

    nc.finalize()
    return nc


def _get_nc():
    if "nc" not in _CACHE:
        _CACHE["nc"] = _build()
    return _CACHE["nc"]


def _make_in_maps(t, x, W1, b1, W2, b2, Wg, bg):
    import ml_dtypes

    bf16 = ml_dtypes.bfloat16
    xT = np.ascontiguousarray(x.T.astype(bf16))
    wgx = np.asarray(Wg[:_D], dtype=np.float32)
    gb = (np.float32(t[0]) * Wg[2 * _D] + bg).astype(np.float32)
    in_maps = []
    for c in range(_NCORES):
        perm = [c] + [e for e in range(_E) if e != c]
        wgp = wgx[:, perm].astype(bf16)  # [D, E]
        b1gb = np.zeros((128, _HT + 1), dtype=np.float32)
        b1gb[:, 0:_HT] = b1[c].reshape(_HT, 128).T
        b1gb[0:_E, _HT] = gb[perm]
        in_maps.append(
            {
                "XT": xT,
                "W1E": np.ascontiguousarray(W1[c].astype(bf16)),
                # SBUF layout [128, HT*D]: row p, col hh*D+d = W2[hh*128+p, d]
                "W2P": np.ascontiguousarray(
                    W2[c].astype(bf16)
                    .reshape(_HT, 128, _D).transpose(1, 0, 2).reshape(128, _HT * _D)
                ),
                "B1GB": b1gb,
                # SBUF layout [128, DT*E]: row p, col d*E+e = wgp[d*128+p, e]
                "WGP": np.ascontiguousarray(
                    wgp.reshape(_DT, 128, _E).transpose(1, 0, 2).reshape(128, _DT * _E)
                ),
            }
        )
    return in_maps


def _assemble(results, inputs):
    # reconstruct softmax denominator and active mask from the exported
    # per-core exp rows; device partials carry the unnormalized E weight
    E = np.stack(
        [np.asarray(results[c]["EROW"]).astype(np.float64).reshape(_B)
         for c in range(_NCORES)]
    )  # [E, B]
    S = E.sum(axis=0)  # [B]
    w = E / S
    out = np.zeros((_B, _D), dtype=np.float64)
    for c in range(_NCORES):
        if (w[c] > _THRESH).any():
            out += results[c]["OUTT"].astype(np.float64).T
    out /= S[:, None]
    b2 = np.asarray(inputs["b2"])
    if np.any(b2):
        # rank-1 bias term sum_e m_e * w[:,e] b2[e,:] — numpy gating replay
        t, x, Wg, bg = (np.asarray(inputs[k]) for k in ("t", "x", "Wg", "bg"))
        logits = x.astype(np.float64) @ Wg[:_D].astype(np.float64)
        logits += np.float64(t[0]) * Wg[2 * _D].astype(np.float64) + bg
        ex = np.exp(logits - logits.max(axis=1, keepdims=True))
        w = ex / ex.sum(axis=1, keepdims=True)
        active = (w > _THRESH).any(axis=0)
        out += (w * active) @ b2.astype(np.float64)
    return out.astype(np.float32)


def run_on_device(t, x, W1, b1, W2, b2, Wg, bg, trace=False):
    from concourse.bass_utils import run_bass_kernel_spmd

    inputs = dict(t=t, x=x, W1=W1, b1=b1, W2=W2, b2=b2, Wg=Wg, bg=bg)
    in_maps = _make_in_maps(**inputs)
    res = run_bass_kernel_spmd(
        _get_nc(), in_maps, list(range(_NCORES)), trace=trace
    )
    return _assemble(res.results, inputs), res


def kernel(t, x, W1, b1, W2, b2, Wg, bg):
    out, _ = run_on_device(t, x, W1, b1, W2, b2, Wg, bg, trace=False)
    return out


# revision 18
# speedup vs baseline: 1.2121x; 1.2121x over previous
"""Trainium2 Bass kernel for nn_ODEFunc_90159953478502 (MoE routing, inference path).

Math (see reference):
    logits  = x @ Wg[:256] + (t*Wg[512] + bg)      # zeros kill Wg[256:512]
    w       = softmax(logits, axis=-1)             # [B, E]
    eo_e    = tanh(x @ W1[e] + b1[e]) @ W2[e] + b2[e]
    active_e = any_b(w[b,e] > 0.01)
    out     = sum_e active_e * w[:,e,None] * eo_e  # softmax max >= 1/8 > 0.01,
                                                   # so >=1 expert always active

Sharding: expert-parallel. Core e holds the full batch plus only W1[e]/W2[e]
and computes the UNNORMALIZED partial E_e[:,None] * (tanh(x@W1[e]+b1[e]) @
W2[e]) in transposed layout ([D, B]), where E_e = exp(logit_e). Because
out = (sum_e m_e * E_e . eo_e) / S shares one softmax denominator S across
experts, normalization and the 0/1 active mask move to the host-side
unshard: each core exports its exp row (EROW), the host reconstructs
S = sum_e E_e, w = E/S, the mask, and divides once. The b2 rank-1 term
(zero here) is added host-side from a numpy gating pass when nonzero.

Device structure per core (all matmuls bf16; warm PE streams one
[128,512] matmul per ~216ns, so the PE floor is ~34 MMs/chunk * 8 chunks
~= 59us; everything else is arranged to keep PE saturated from ~3us on):
  - x arrives pre-transposed (xT [D, B] bf16) so W1/W2/Wg act as matmul lhsT
    in natural layout (out = lhsT.T @ rhs, contraction on partitions).
  - W2 and Wg are prepacked on the HOST into their exact SBUF layouts
    ([128, HT*D] / [128, DT*E]) so their DMAs are few large contiguous
    row-descriptors instead of thousands of 16-512B ones.
  - DMA issue plan (per-engine FIFO; SP + Act HWDGE rings + gpsimd SWDGE):
      sync  : x_d0_c0, x_d1_c0, w1_d0, x_d0_c1, x_d0_c23, x_d0_c47,
              batched mm2 stores, final d2=0 store
      scalar: wgx, gb, b1, w1_d1, w2, EROW, final d2=1 store
      gpsimd: x_d1_c1, x_d1_c23, x_d1_c47 (off the critical path)
    so gating(0) deps (x c0 + gating consts) land ~2.5us in, mm1(0) deps
    right behind, and no critical load queues behind a megabyte transfer.
  - a short junk-matmul warmup stream (DVE-memset source, PSUM sink shared
    with the mm2 pool) trips the PE HAM clock gate during the DMA wait so
    real matmuls run at 2.4GHz nearly from the start.
  - gating weights are column-permuted per core so the core's own expert is
    partition row 0 of the [E, B] logits: no selector matmuls needed.
  - per 512-chunk: logits^T [8,512] via 2 bf16 matmuls, ACT Exp with fused
    +gbias (no max-subtract: |logits| <= ~4) emitting bf16 into a slice of
    e_all [8, B]; chunk-pair rows are gpsimd partition-broadcast to wb
    [128,1024] bf16 (no DRAM bounce, no S-matmul, no reciprocal).
  - mm2 drains: out^T tile = psum * wb, one DVE multiply per [128,512] tile;
    stores batch 2 chunks per descriptor except the final pair (small
    descriptors keep the post-compute DMA tail short).
"""

import sys

if "/opt/trn_rl_repo" not in sys.path:
    sys.path.insert(0, "/opt/trn_rl_repo")

import numpy as np

_B, _D, _H, _E = 4096, 256, 1024, 8
_NCORES = 8
_CHUNK = 512
_NCH = _B // _CHUNK
_DT = _D // 128   # 2 d-tiles
_HT = _H // 128   # 8 h-tiles
_THRESH = 0.01
_NWARM = 12       # junk warmup matmuls

_CACHE = {}


def _build():
    import concourse.bass as bass
    import concourse.tile as tile
    import concourse.mybir as mybir
    from concourse import bacc
    from contextlib import ExitStack

    F32 = mybir.dt.float32
    BF16 = mybir.dt.bfloat16
    AF = mybir.ActivationFunctionType
    ALU = mybir.AluOpType

    nc = bacc.Bacc("TRN2", target_bir_lowering=False, debug=False)

    XT = nc.declare_dram_parameter("XT", [_D, _B], BF16, isOutput=False)
    W1E = nc.declare_dram_parameter("W1E", [_D, _H], BF16, isOutput=False)
    # host-prepacked to SBUF layout: W2P[p, hh*D+d] = W2[hh*128+p, d]
    W2P = nc.declare_dram_parameter("W2P", [128, _HT * _D], BF16, isOutput=False)
    # b1 in SBUF layout [128, HT] with gb (t*Wg[2D]+bg, permuted) as col HT
    B1GB = nc.declare_dram_parameter("B1GB", [128, _HT + 1], F32, isOutput=False)
    # host-prepacked: WGP[p, d*E+e] = Wg[d*128+p, perm[e]]
    WGP = nc.declare_dram_parameter("WGP", [128, _DT * _E], BF16, isOutput=False)
    OUTT = nc.declare_dram_parameter("OUTT", [_D, _B], BF16, isOutput=True)
    EROW = nc.declare_dram_parameter("EROW", [1, _B], BF16, isOutput=True)

    with tile.TileContext(nc) as tc, ExitStack() as ctx:
        const = ctx.enter_context(tc.tile_pool(name="const", bufs=1))
        wbp = ctx.enter_context(tc.tile_pool(name="wbp", bufs=4))
        htp = ctx.enter_context(tc.tile_pool(name="htp", bufs=24))
        op = ctx.enter_context(tc.tile_pool(name="op", bufs=4))
        pg = ctx.enter_context(tc.tile_pool(name="pg", bufs=2, space="PSUM"))
        ph = ctx.enter_context(tc.tile_pool(name="ph", bufs=4, space="PSUM"))
        po = ctx.enter_context(tc.tile_pool(name="po", bufs=2, space="PSUM"))

        # ---- PE warmup: a short junk-matmul stream with no DMA deps trips
        # the HAM clock gate (4096-cycle busy window) while inputs load, so
        # the real matmul stream starts at 2.4GHz. Source tile is memset on
        # the otherwise-idle gpsimd (no uninitialized reads); sink shares
        # the mm2 PSUM pool (its slot is recycled long after the junk
        # stream retires).
        junk = const.tile([128, 128 + _CHUNK], BF16)
        nc.gpsimd.memset(junk[:], 0.0)
        pjunk = po.tile([128, _CHUNK], F32, tag="pso")
        for _ in range(_NWARM):
            nc.tensor.matmul(
                pjunk[:], junk[:, 0:128], junk[:, 128 : 128 + _CHUNK],
                start=True, stop=True,
            )

        # ---- inputs, two phases. The 16 SDMA engines round-robin fairly
        # over every QUEUED transfer, so a critical 128KB load queued next
        # to megabytes of bulk only gets its fair share. Phase 1 queues
        # just the chunk-0/1 working set (x c0/c1, gating consts, w1) over
        # three DGE paths. Phase 2 (w2, x c2..c7) is held back by a
        # tile-visible WAW: a 1-column DVE copy from the last phase-1 tile
        # into each phase-2 destination, so its descriptors only enter the
        # rings once phase 1 has landed. -----------------------------------
        xd = [
            const.tile([128, _B], BF16, tag=f"xd_{d}", name=f"xd_{d}")
            for d in range(_DT)
        ]
        xm = {(d, c): xd[d][:, c * _CHUNK : (c + 1) * _CHUNK]
              for d in range(_DT) for c in range(_NCH)}
        w1 = [
            const.tile([128, _H], BF16, tag=f"w1_{d}", name=f"w1_{d}")
            for d in range(_DT)
        ]

        # descriptor size = per-partition contiguous bytes, and it sets the
        # effective HBM rate (~190GB/s at 1KB rows vs ~300GB/s at 2-8KB), so
        # x loads are 1024-column (2KB) pieces and w1 whole tiles (2KB).
        # phase 1 — SP ring
        nc.sync.dma_start(xd[0][:, 0 : 2 * _CHUNK], XT.ap()[0:128, 0 : 2 * _CHUNK])
        nc.sync.dma_start(w1[0][:], W1E.ap()[0:128, :])
        # phase 1 — Act ring (xd1 c01 leads so gating(0)'s d1 matmul and
        # Exp unblock ASAP; nothing else EVER queues DMA here before the
        # Exp/Tanh stream — a DIRECT2D is ~650ns of sequencer time and
        # stalls ACT issue)
        wgx_sb = const.tile([128, _DT * _E], BF16)
        nc.scalar.dma_start(wgx_sb[:], WGP.ap())
        b1gb = const.tile([128, _HT + 1], F32)
        nc.scalar.dma_start(b1gb[:], B1GB.ap())
        b1_sb = b1gb[:, 0:_HT]
        gb_sb = b1gb[0:_E, _HT : _HT + 1]
        nc.scalar.dma_start(w1[1][:], W1E.ap()[128:256, :])
        nc.scalar.dma_start(
            xd[1][:, 0 : 2 * _CHUNK], XT.ap()[128:256, 0 : 2 * _CHUNK]
        )

        # phase 2 on the sync + gpsimd queues only (no early engine work
        # there), gated on the last phase-1 transfer via WAW slivers
        w2_all = const.tile([128, _HT * _D], BF16)
        gate_src = xd[1][:, 2 * _CHUNK - 1 : 2 * _CHUNK]
        for dst in (
            w2_all[:, 0:1],
            xd[0][:, 2 * _CHUNK : 2 * _CHUNK + 1],
            xd[1][:, 2 * _CHUNK : 2 * _CHUNK + 1],
            xd[0][:, _B // 2 : _B // 2 + 1],
            xd[1][:, _B // 2 : _B // 2 + 1],
        ):
            nc.vector.tensor_copy(dst, gate_src)
        nc.sync.dma_start(
            xd[0][:, 2 * _CHUNK : _B // 2],
            XT.ap()[0:128, 2 * _CHUNK : _B // 2],
        )
        nc.sync.dma_start(w2_all[:], W2P.ap())
        nc.sync.dma_start(xd[0][:, _B // 2 : _B], XT.ap()[0:128, _B // 2 : _B])
        nc.gpsimd.dma_start(
            xd[1][:, 2 * _CHUNK : _B // 2],
            XT.ap()[128:256, 2 * _CHUNK : _B // 2],
        )
        nc.gpsimd.dma_start(xd[1][:, _B // 2 : _B], XT.ap()[128:256, _B // 2 : _B])

        # ---- gating chunk: unnormalized own-expert exp row -> wb broadcast.
        # Softmax normalization and the active mask move to the host-side
        # unshard: out = (sum_e m_e * (E_e . eo_e)) / S shares one
        # denominator S across experts, and each core exports its exp row
        # (EROW), from which the host reconstructs S, w, and the mask.
        e_all = const.tile([_E, _B], BF16)
        wb_tiles = {}

        def gating(c):
            psg = pg.tile([_E, _CHUNK], F32, tag="pg")
            for d in range(_DT):
                nc.tensor.matmul(
                    psg[:], wgx_sb[:, d * _E : (d + 1) * _E], xm[(d, c)],
                    start=(d == 0), stop=(d == _DT - 1),
                )
            cs = slice(c * _CHUNK, (c + 1) * _CHUNK)
            nc.scalar.activation(e_all[:, cs], psg[:], AF.Exp, bias=gb_sb[:])
            if c % 2 == 1:
                ps2 = slice((c - 1) * _CHUNK, (c + 1) * _CHUNK)
                wb = wbp.tile([128, 2 * _CHUNK], BF16, tag="wb")
                nc.gpsimd.partition_broadcast(wb[:], e_all[0:1, ps2], 128)
                wb_tiles[c - 1] = wb[:, 0:_CHUNK]
                wb_tiles[c] = wb[:, _CHUNK : 2 * _CHUNK]

        # ---- main, software-pipelined on PE: mm1(c+1) precedes mm2(c) ------
        ht_by_chunk = {}

        def mm1(c):
            ht_tiles = []
            for hh in range(_HT):
                psh = ph.tile([128, _CHUNK], F32, tag="psh")
                for d in range(_DT):
                    nc.tensor.matmul(
                        psh[:],
                        w1[d][:, hh * 128 : (hh + 1) * 128],
                        xm[(d, c)],
                        start=(d == 0), stop=(d == _DT - 1),
                    )
                ht = htp.tile([128, _CHUNK], BF16, tag="ht")
                nc.scalar.activation(
                    ht[:], psh[:], AF.Tanh, bias=b1_sb[:, hh : hh + 1]
                )
                ht_tiles.append(ht)
            ht_by_chunk[c] = ht_tiles

        obuf = {}

        def mm2(c):
            # outputs batch 2 chunks per descriptor except the final pair:
            # small per-chunk descriptors there keep the post-compute DMA
            # tail short (a trailing 1MB transfer costs ~2.6us before the
            # teardown barrier can pass).
            batch = c < _NCH - 2
            half = c % 2
            ht_tiles = ht_by_chunk.pop(c)
            for d2 in range(_DT):
                pso = po.tile([128, _CHUNK], F32, tag="pso")
                for hh in range(_HT):
                    nc.tensor.matmul(
                        pso[:],
                        w2_all[:, hh * _D + d2 * 128 : hh * _D + (d2 + 1) * 128],
                        ht_tiles[hh][:],
                        start=(hh == 0), stop=(hh == _HT - 1),
                    )
                if batch:
                    if half == 0:
                        osb_t = op.tile([128, 2 * _CHUNK], BF16, tag="osb")
                        obuf[d2] = osb_t
                    osb = obuf[d2]
                    nc.vector.tensor_tensor(
                        osb[:, half * _CHUNK : (half + 1) * _CHUNK],
                        pso[:], wb_tiles[c], ALU.mult,
                    )
                    if half == 1:
                        nc.sync.dma_start(
                            OUTT.ap()[
                                d2 * 128 : (d2 + 1) * 128,
                                (c - 1) * _CHUNK : (c + 1) * _CHUNK,
                            ],
                            osb[:],
                        )
                else:
                    osb_t = op.tile([128, _CHUNK], BF16, tag="osb1")
                    nc.vector.tensor_tensor(
                        osb_t[:], pso[:], wb_tiles[c], ALU.mult
                    )
                    eng = nc.sync if d2 == 0 else nc.scalar
                    eng.dma_start(
                        OUTT.ap()[
                            d2 * 128 : (d2 + 1) * 128,
                            c * _CHUNK : (c + 1) * _CHUNK,
                        ],
                        osb_t[:],
                    )

        gating(0)
        mm1(0)
        gating(1)
        mm1(1)
        for c in range(_NCH):
            if c + 2 < _NCH:
                gating(c + 2)
                if c + 2 == _NCH - 1:
                    # EROW only needs the last Exp; issuing it here keeps it
                    # off the ring tails behind the final stores
                    nc.scalar.dma_start(EROW.ap()[0:1, :], e_all[0:1, :])
            if c + 2 < _NCH:
                mm1(c + 2)
            mm2(c)

    nc.finalize()
    return nc


def _get_nc():
    if "nc" not in _CACHE:
        _CACHE["nc"] = _build()
    return _CACHE["nc"]


def _make_in_maps(t, x, W1, b1, W2, b2, Wg, bg):
    import ml_dtypes

    bf16 = ml_dtypes.bfloat16
    xT = np.ascontiguousarray(x.T.astype(bf16))
    wgx = np.asarray(Wg[:_D], dtype=np.float32)
    gb = (np.float32(t[0]) * Wg[2 * _D] + bg).astype(np.float32)
    in_maps = []
    for c in range(_NCORES):
        perm = [c] + [e for e in range(_E) if e != c]
        wgp = wgx[:, perm].astype(bf16)  # [D, E]
        b1gb = np.zeros((128, _HT + 1), dtype=np.float32)
        b1gb[:, 0:_HT] = b1[c].reshape(_HT, 128).T
        b1gb[0:_E, _HT] = gb[perm]
        in_maps.append(
            {
                "XT": xT,
                "W1E": np.ascontiguousarray(W1[c].astype(bf16)),
                # SBUF layout [128, HT*D]: row p, col hh*D+d = W2[hh*128+p, d]
                "W2P": np.ascontiguousarray(
                    W2[c].astype(bf16)
                    .reshape(_HT, 128, _D).transpose(1, 0, 2).reshape(128, _HT * _D)
                ),
                "B1GB": b1gb,
                # SBUF layout [128, DT*E]: row p, col d*E+e = wgp[d*128+p, e]
                "WGP": np.ascontiguousarray(
                    wgp.reshape(_DT, 128, _E).transpose(1, 0, 2).reshape(128, _DT * _E)
                ),
            }
        )
    return in_maps


def _assemble(results, inputs):
    # reconstruct softmax denominator and active mask from the exported
    # per-core exp rows; device partials carry the unnormalized E weight
    E = np.stack(
        [np.asarray(results[c]["EROW"]).astype(np.float64).reshape(_B)
         for c in range(_NCORES)]
    )  # [E, B]
    S = E.sum(axis=0)  # [B]
    w = E / S
    out = np.zeros((_B, _D), dtype=np.float64)
    for c in range(_NCORES):
        if (w[c] > _THRESH).any():
            out += results[c]["OUTT"].astype(np.float64).T
    out /= S[:, None]
    b2 = np.asarray(inputs["b2"])
    if np.any(b2):
        # rank-1 bias term sum_e m_e * w[:,e] b2[e,:] — numpy gating replay
        t, x, Wg, bg = (np.asarray(inputs[k]) for k in ("t", "x", "Wg", "bg"))
        logits = x.astype(np.float64) @ Wg[:_D].astype(np.float64)
        logits += np.float64(t[0]) * Wg[2 * _D].astype(np.float64) + bg
        ex = np.exp(logits - logits.max(axis=1, keepdims=True))
        w = ex / ex.sum(axis=1, keepdims=True)
        active = (w > _THRESH).any(axis=0)
        out += (w * active) @ b2.astype(np.float64)
    return out.astype(np.float32)


def run_on_device(t, x, W1, b1, W2, b2, Wg, bg, trace=False):
    from concourse.bass_utils import run_bass_kernel_spmd

    inputs = dict(t=t, x=x, W1=W1, b1=b1, W2=W2, b2=b2, Wg=Wg, bg=bg)
    in_maps = _make_in_maps(**inputs)
    res = run_bass_kernel_spmd(
        _get_nc(), in_maps, list(range(_NCORES)), trace=trace
    )
    return _assemble(res.results, inputs), res


def kernel(t, x, W1, b1, W2, b2, Wg, bg):
    out, _ = run_on_device(t, x, W1, b1, W2, b2, Wg, bg, trace=False)
    return out


# revision 19
# speedup vs baseline: 1.2293x; 1.0142x over previous
"""Trainium2 Bass kernel for nn_ODEFunc_90159953478502 (MoE routing, inference path).

Math (see reference):
    logits  = x @ Wg[:256] + (t*Wg[512] + bg)      # zeros kill Wg[256:512]
    w       = softmax(logits, axis=-1)             # [B, E]
    eo_e    = tanh(x @ W1[e] + b1[e]) @ W2[e] + b2[e]
    active_e = any_b(w[b,e] > 0.01)
    out     = sum_e active_e * w[:,e,None] * eo_e  # softmax max >= 1/8 > 0.01,
                                                   # so >=1 expert always active

Sharding: expert-parallel. Core e holds the full batch plus only W1[e]/W2[e]
and computes the UNNORMALIZED partial E_e[:,None] * (tanh(x@W1[e]+b1[e]) @
W2[e]) in transposed layout ([D, B]), where E_e = exp(logit_e). Because
out = (sum_e m_e * E_e . eo_e) / S shares one softmax denominator S across
experts, normalization and the 0/1 active mask move to the host-side
unshard: each core exports its exp row (EROW), the host reconstructs
S = sum_e E_e, w = E/S, the mask, and divides once. The b2 rank-1 term
(zero here) is added host-side from a numpy gating pass when nonzero.

Device structure per core (all matmuls bf16; warm PE streams one
[128,512] matmul per ~216ns, so the PE floor is ~34 MMs/chunk * 8 chunks
~= 59us; everything else is arranged to keep PE saturated from ~3us on):
  - x arrives pre-transposed (xT [D, B] bf16) so W1/W2/Wg act as matmul lhsT
    in natural layout (out = lhsT.T @ rhs, contraction on partitions).
  - W2 and Wg are prepacked on the HOST into their exact SBUF layouts
    ([128, HT*D] / [128, DT*E]) so their DMAs are few large contiguous
    row-descriptors instead of thousands of 16-512B ones.
  - DMA issue plan (per-engine FIFO; SP + Act HWDGE rings + gpsimd SWDGE):
      sync  : x_d0_c0, x_d1_c0, w1_d0, x_d0_c1, x_d0_c23, x_d0_c47,
              batched mm2 stores, final d2=0 store
      scalar: wgx, gb, b1, w1_d1, w2, EROW, final d2=1 store
      gpsimd: x_d1_c1, x_d1_c23, x_d1_c47 (off the critical path)
    so gating(0) deps (x c0 + gating consts) land ~2.5us in, mm1(0) deps
    right behind, and no critical load queues behind a megabyte transfer.
  - a short junk-matmul warmup stream (DVE-memset source, PSUM sink shared
    with the mm2 pool) trips the PE HAM clock gate during the DMA wait so
    real matmuls run at 2.4GHz nearly from the start.
  - gating weights are column-permuted per core so the core's own expert is
    partition row 0 of the [E, B] logits: no selector matmuls needed.
  - per 512-chunk: logits^T [8,512] via 2 bf16 matmuls, ACT Exp with fused
    +gbias (no max-subtract: |logits| <= ~4) emitting bf16 into a slice of
    e_all [8, B]; chunk-pair rows are gpsimd partition-broadcast to wb
    [128,1024] bf16 (no DRAM bounce, no S-matmul, no reciprocal).
  - mm2 drains: out^T tile = psum * wb, one DVE multiply per [128,512] tile;
    stores batch 2 chunks per descriptor except the final pair (small
    descriptors keep the post-compute DMA tail short).
"""

import sys

if "/opt/trn_rl_repo" not in sys.path:
    sys.path.insert(0, "/opt/trn_rl_repo")

import numpy as np

_B, _D, _H, _E = 4096, 256, 1024, 8
_NCORES = 8
_CHUNK = 512
_NCH = _B // _CHUNK
_DT = _D // 128   # 2 d-tiles
_HT = _H // 128   # 8 h-tiles
_THRESH = 0.01
_NWARM = 12       # junk warmup matmuls

_CACHE = {}


def _build():
    import concourse.bass as bass
    import concourse.tile as tile
    import concourse.mybir as mybir
    from concourse import bacc
    from contextlib import ExitStack

    F32 = mybir.dt.float32
    BF16 = mybir.dt.bfloat16
    AF = mybir.ActivationFunctionType
    ALU = mybir.AluOpType

    nc = bacc.Bacc("TRN2", target_bir_lowering=False, debug=False)

    XT = nc.declare_dram_parameter("XT", [_D, _B], BF16, isOutput=False)
    W1E = nc.declare_dram_parameter("W1E", [_D, _H], BF16, isOutput=False)
    # host-prepacked to SBUF layout: W2P[p, hh*D+d] = W2[hh*128+p, d]
    W2P = nc.declare_dram_parameter("W2P", [128, _HT * _D], BF16, isOutput=False)
    # b1 in SBUF layout [128, HT] with gb (t*Wg[2D]+bg, permuted) as col HT
    B1GB = nc.declare_dram_parameter("B1GB", [128, _HT + 1], F32, isOutput=False)
    # host-prepacked, zero-padded to 128 output cols so the gating matmul
    # runs the full-width fast weight-load path: WGP[p, d*128+e] =
    # Wg[d*128+p, perm[e]] for e<8, else 0
    WGP = nc.declare_dram_parameter("WGP", [128, _DT * 128], BF16, isOutput=False)
    OUTT = nc.declare_dram_parameter("OUTT", [_D, _B], BF16, isOutput=True)
    EROW = nc.declare_dram_parameter("EROW", [1, _B], BF16, isOutput=True)

    with tile.TileContext(nc) as tc, ExitStack() as ctx:
        const = ctx.enter_context(tc.tile_pool(name="const", bufs=1))
        wbp = ctx.enter_context(tc.tile_pool(name="wbp", bufs=4))
        htp = ctx.enter_context(tc.tile_pool(name="htp", bufs=24))
        op = ctx.enter_context(tc.tile_pool(name="op", bufs=4))
        pg = ctx.enter_context(tc.tile_pool(name="pg", bufs=2, space="PSUM"))
        ph = ctx.enter_context(tc.tile_pool(name="ph", bufs=4, space="PSUM"))
        po = ctx.enter_context(tc.tile_pool(name="po", bufs=2, space="PSUM"))

        # ---- PE warmup: a short junk-matmul stream with no DMA deps trips
        # the HAM clock gate (4096-cycle busy window) while inputs load, so
        # the real matmul stream starts at 2.4GHz. Source tile is memset on
        # the otherwise-idle gpsimd (no uninitialized reads); sink shares
        # the mm2 PSUM pool (its slot is recycled long after the junk
        # stream retires).
        junk = const.tile([128, 128 + _CHUNK], BF16)
        nc.gpsimd.memset(junk[:], 0.0)
        pjunk = po.tile([128, _CHUNK], F32, tag="pso")
        for _ in range(_NWARM):
            nc.tensor.matmul(
                pjunk[:], junk[:, 0:128], junk[:, 128 : 128 + _CHUNK],
                start=True, stop=True,
            )

        # ---- inputs, two phases. The 16 SDMA engines round-robin fairly
        # over every QUEUED transfer, so a critical 128KB load queued next
        # to megabytes of bulk only gets its fair share. Phase 1 queues
        # just the chunk-0/1 working set (x c0/c1, gating consts, w1) over
        # three DGE paths. Phase 2 (w2, x c2..c7) is held back by a
        # tile-visible WAW: a 1-column DVE copy from the last phase-1 tile
        # into each phase-2 destination, so its descriptors only enter the
        # rings once phase 1 has landed. -----------------------------------
        xd = [
            const.tile([128, _B], BF16, tag=f"xd_{d}", name=f"xd_{d}")
            for d in range(_DT)
        ]
        xm = {(d, c): xd[d][:, c * _CHUNK : (c + 1) * _CHUNK]
              for d in range(_DT) for c in range(_NCH)}
        w1 = [
            const.tile([128, _H], BF16, tag=f"w1_{d}", name=f"w1_{d}")
            for d in range(_DT)
        ]

        # descriptor size = per-partition contiguous bytes, and it sets the
        # effective HBM rate (~190GB/s at 1KB rows vs ~300GB/s at 2-8KB), so
        # x loads are 1024-column (2KB) pieces and w1 whole tiles (2KB).
        # phase 1 — SP ring
        nc.sync.dma_start(xd[0][:, 0 : 2 * _CHUNK], XT.ap()[0:128, 0 : 2 * _CHUNK])
        nc.sync.dma_start(w1[0][:], W1E.ap()[0:128, :])
        # phase 1 — Act ring (xd1 c01 leads so gating(0)'s d1 matmul and
        # Exp unblock ASAP; nothing else EVER queues DMA here before the
        # Exp/Tanh stream — a DIRECT2D is ~650ns of sequencer time and
        # stalls ACT issue)
        wgx_sb = const.tile([128, _DT * 128], BF16)
        nc.scalar.dma_start(wgx_sb[:], WGP.ap())
        b1gb = const.tile([128, _HT + 1], F32)
        nc.scalar.dma_start(b1gb[:], B1GB.ap())
        b1_sb = b1gb[:, 0:_HT]
        gb_sb = b1gb[0:_E, _HT : _HT + 1]
        nc.scalar.dma_start(w1[1][:], W1E.ap()[128:256, :])
        nc.scalar.dma_start(
            xd[1][:, 0 : 2 * _CHUNK], XT.ap()[128:256, 0 : 2 * _CHUNK]
        )

        # phase 2 on the sync + gpsimd queues only (no early engine work
        # there), gated on the last phase-1 transfer via WAW slivers
        w2_all = const.tile([128, _HT * _D], BF16)
        gate_src = xd[1][:, 2 * _CHUNK - 1 : 2 * _CHUNK]
        for dst in (
            w2_all[:, 0:1],
            xd[0][:, 2 * _CHUNK : 2 * _CHUNK + 1],
            xd[1][:, 2 * _CHUNK : 2 * _CHUNK + 1],
            xd[0][:, _B // 2 : _B // 2 + 1],
            xd[1][:, _B // 2 : _B // 2 + 1],
        ):
            nc.vector.tensor_copy(dst, gate_src)
        nc.sync.dma_start(
            xd[0][:, 2 * _CHUNK : _B // 2],
            XT.ap()[0:128, 2 * _CHUNK : _B // 2],
        )
        nc.sync.dma_start(w2_all[:], W2P.ap())
        nc.sync.dma_start(xd[0][:, _B // 2 : _B], XT.ap()[0:128, _B // 2 : _B])
        nc.gpsimd.dma_start(
            xd[1][:, 2 * _CHUNK : _B // 2],
            XT.ap()[128:256, 2 * _CHUNK : _B // 2],
        )
        nc.gpsimd.dma_start(xd[1][:, _B // 2 : _B], XT.ap()[128:256, _B // 2 : _B])

        # ---- gating chunk: unnormalized own-expert exp row -> wb broadcast.
        # Softmax normalization and the active mask move to the host-side
        # unshard: out = (sum_e m_e * (E_e . eo_e)) / S shares one
        # denominator S across experts, and each core exports its exp row
        # (EROW), from which the host reconstructs S, w, and the mask.
        e_all = const.tile([_E, _B], BF16)
        wb_tiles = {}

        psg_by_chunk = {}

        def gating_mm(c):
            psg = pg.tile([128, _CHUNK], F32, tag="pg")
            for d in range(_DT):
                nc.tensor.matmul(
                    psg[:], wgx_sb[:, d * 128 : (d + 1) * 128], xm[(d, c)],
                    start=(d == 0), stop=(d == _DT - 1),
                )
            psg_by_chunk[c] = psg

        def gating_exp(c):
            # emitted AFTER mm1(c) so the chunk's Tanhs aren't stuck in the
            # ACT FIFO behind an Exp whose psg matmul the PE scheduler
            # defers (that logjam stalled mm1(c+1) on PSUM banks)
            psg = psg_by_chunk.pop(c)
            cs = slice(c * _CHUNK, (c + 1) * _CHUNK)
            nc.scalar.activation(e_all[:, cs], psg[0:_E, :], AF.Exp, bias=gb_sb[:])
            if c % 2 == 1:
                ps2 = slice((c - 1) * _CHUNK, (c + 1) * _CHUNK)
                wb = wbp.tile([128, 2 * _CHUNK], BF16, tag="wb")
                nc.gpsimd.partition_broadcast(wb[:], e_all[0:1, ps2], 128)
                wb_tiles[c - 1] = wb[:, 0:_CHUNK]
                wb_tiles[c] = wb[:, _CHUNK : 2 * _CHUNK]

        # ---- main, software-pipelined on PE: mm1(c+1) precedes mm2(c) ------
        ht_by_chunk = {}

        def mm1(c):
            ht_tiles = []
            for hh in range(_HT):
                psh = ph.tile([128, _CHUNK], F32, tag="psh")
                for d in range(_DT):
                    nc.tensor.matmul(
                        psh[:],
                        w1[d][:, hh * 128 : (hh + 1) * 128],
                        xm[(d, c)],
                        start=(d == 0), stop=(d == _DT - 1),
                    )
                ht = htp.tile([128, _CHUNK], BF16, tag="ht")
                nc.scalar.activation(
                    ht[:], psh[:], AF.Tanh, bias=b1_sb[:, hh : hh + 1]
                )
                ht_tiles.append(ht)
            ht_by_chunk[c] = ht_tiles

        obuf = {}

        def mm2(c):
            # outputs batch 2 chunks per descriptor except the final pair:
            # small per-chunk descriptors there keep the post-compute DMA
            # tail short (a trailing 1MB transfer costs ~2.6us before the
            # teardown barrier can pass).
            batch = c < _NCH - 2
            half = c % 2
            ht_tiles = ht_by_chunk.pop(c)
            for d2 in range(_DT):
                pso = po.tile([128, _CHUNK], F32, tag="pso")
                for hh in range(_HT):
                    nc.tensor.matmul(
                        pso[:],
                        w2_all[:, hh * _D + d2 * 128 : hh * _D + (d2 + 1) * 128],
                        ht_tiles[hh][:],
                        start=(hh == 0), stop=(hh == _HT - 1),
                    )
                if batch:
                    if half == 0:
                        osb_t = op.tile([128, 2 * _CHUNK], BF16, tag="osb")
                        obuf[d2] = osb_t
                    osb = obuf[d2]
                    nc.vector.tensor_tensor(
                        osb[:, half * _CHUNK : (half + 1) * _CHUNK],
                        pso[:], wb_tiles[c], ALU.mult,
                    )
                    if half == 1:
                        nc.sync.dma_start(
                            OUTT.ap()[
                                d2 * 128 : (d2 + 1) * 128,
                                (c - 1) * _CHUNK : (c + 1) * _CHUNK,
                            ],
                            osb[:],
                        )
                else:
                    osb_t = op.tile([128, _CHUNK], BF16, tag="osb1")
                    nc.vector.tensor_tensor(
                        osb_t[:], pso[:], wb_tiles[c], ALU.mult
                    )
                    eng = nc.sync if d2 == 0 else nc.scalar
                    eng.dma_start(
                        OUTT.ap()[
                            d2 * 128 : (d2 + 1) * 128,
                            c * _CHUNK : (c + 1) * _CHUNK,
                        ],
                        osb_t[:],
                    )

        gating_mm(0)
        mm1(0)
        gating_exp(0)
        gating_mm(1)
        mm1(1)
        gating_exp(1)
        for c in range(_NCH):
            if c + 2 < _NCH:
                gating_mm(c + 2)
                mm1(c + 2)
                gating_exp(c + 2)
                if c + 2 == _NCH - 1:
                    # EROW only needs the last Exp; sync queue so its
                    # DIRECT2D never stalls ACT issue
                    nc.sync.dma_start(EROW.ap()[0:1, :], e_all[0:1, :])
            mm2(c)

    nc.finalize()
    return nc


def _get_nc():
    if "nc" not in _CACHE:
        _CACHE["nc"] = _build()
    return _CACHE["nc"]


def _pack_wgp(wgp):
    pad = np.zeros((_D, 128), dtype=wgp.dtype)
    pad[:, 0:_E] = wgp
    return np.ascontiguousarray(
        pad.reshape(_DT, 128, 128).transpose(1, 0, 2).reshape(128, _DT * 128)
    )


def _make_in_maps(t, x, W1, b1, W2, b2, Wg, bg):
    import ml_dtypes

    bf16 = ml_dtypes.bfloat16
    xT = np.ascontiguousarray(x.T.astype(bf16))
    wgx = np.asarray(Wg[:_D], dtype=np.float32)
    gb = (np.float32(t[0]) * Wg[2 * _D] + bg).astype(np.float32)
    in_maps = []
    for c in range(_NCORES):
        perm = [c] + [e for e in range(_E) if e != c]
        wgp = wgx[:, perm].astype(bf16)  # [D, E]
        b1gb = np.zeros((128, _HT + 1), dtype=np.float32)
        b1gb[:, 0:_HT] = b1[c].reshape(_HT, 128).T
        b1gb[0:_E, _HT] = gb[perm]
        in_maps.append(
            {
                "XT": xT,
                "W1E": np.ascontiguousarray(W1[c].astype(bf16)),
                # SBUF layout [128, HT*D]: row p, col hh*D+d = W2[hh*128+p, d]
                "W2P": np.ascontiguousarray(
                    W2[c].astype(bf16)
                    .reshape(_HT, 128, _D).transpose(1, 0, 2).reshape(128, _HT * _D)
                ),
                "B1GB": b1gb,
                # SBUF layout [128, DT*128], zero-padded past col 8 of
                # each d-group: row p, col d*128+e = wgp[d*128+p, e]
                "WGP": _pack_wgp(wgp),
            }
        )
    return in_maps


def _assemble(results, inputs):
    # reconstruct softmax denominator and active mask from the exported
    # per-core exp rows; device partials carry the unnormalized E weight
    E = np.stack(
        [np.asarray(results[c]["EROW"]).astype(np.float64).reshape(_B)
         for c in range(_NCORES)]
    )  # [E, B]
    S = E.sum(axis=0)  # [B]
    w = E / S
    out = np.zeros((_B, _D), dtype=np.float64)
    for c in range(_NCORES):
        if (w[c] > _THRESH).any():
            out += results[c]["OUTT"].astype(np.float64).T
    out /= S[:, None]
    b2 = np.asarray(inputs["b2"])
    if np.any(b2):
        # rank-1 bias term sum_e m_e * w[:,e] b2[e,:] — numpy gating replay
        t, x, Wg, bg = (np.asarray(inputs[k]) for k in ("t", "x", "Wg", "bg"))
        logits = x.astype(np.float64) @ Wg[:_D].astype(np.float64)
        logits += np.float64(t[0]) * Wg[2 * _D].astype(np.float64) + bg
        ex = np.exp(logits - logits.max(axis=1, keepdims=True))
        w = ex / ex.sum(axis=1, keepdims=True)
        active = (w > _THRESH).any(axis=0)
        out += (w * active) @ b2.astype(np.float64)
    return out.astype(np.float32)


def run_on_device(t, x, W1, b1, W2, b2, Wg, bg, trace=False):
    from concourse.bass_utils import run_bass_kernel_spmd

    inputs = dict(t=t, x=x, W1=W1, b1=b1, W2=W2, b2=b2, Wg=Wg, bg=bg)
    in_maps = _make_in_maps(**inputs)
    res = run_bass_kernel_spmd(
        _get_nc(), in_maps, list(range(_NCORES)), trace=trace
    )
    return _assemble(res.results, inputs), res


def kernel(t, x, W1, b1, W2, b2, Wg, bg):
    out, _ = run_on_device(t, x, W1, b1, W2, b2, Wg, bg, trace=False)
    return out
